# revision 30
# baseline (speedup 1.0000x reference)
"""DIEN (GRU + AUGRU scan) Trainium2 Bass kernel — v2 split-bank pipeline.

Strategy
--------
Data-parallel over batch: B=256 split 8 ways (32 per core); weights replicated;
the T=200 scan runs locally per core. Attention v_proj folded into the AUGRU
input weights (exact; per-step softmax over length-1 sequence == 1).

v2 changes vs the prewrite/single-bank version:
- Each cell's psum is split across TWO banks: A = [gin | r | z] (x-side +
  hidden rz accumulate there), B = [hn] alone. PSUM bank R/W collisions are
  fatal and Tile orders conservatively per bank, so with hn in its own bank
  the sigmoid over [r|z] can run while the hn matmuls still stream.
- gin is read straight out of PSUM by the t3 add on DVE (no ACT ugin copy).
- All psum biases are engine-prewritten off the critical path from
  host-broadcast tiles: A banks on ACT ([128,384], one step ahead), B banks
  on DVE ([128,128], two steps ahead). K=1 bias matmuls were measured to
  cost ~1us/step of PE (LDWEIGHTS churn against the 32-col state slices) —
  engine prewrites are far cheaper.
- Post-tanh tail runs in bf16 on DVE (2x mode); sigmoid outputs fp32 (r
  feeds the fp32 v mul against PSUM).
- Input DMAs are split across the SP and ACT HWDGE queues so the prologue
  weight transfers overlap ~2x (saves ~0.7 ms on a cold single execution).

Per-step engine budget (HW model, ns): PE ~2850 (4 GEMM groups, 4x column
tiling, + 8 K=1 bias mms), ACT ~1810, DVE ~2420, POOL ~1940. The serial
g-recurrence (ghid rz -> sig -> v -> t3 -> tanh -> m1 -> row -> transpose)
fits inside the PE period with ~250ns slack.

Layouts (per core, batch b 0..31, hidden h = 128*c + 32*m + jr):
  row layout  : tile[32*c + b, 32*m + jr]  (states, psum outputs)
  stationary  : tileT[32*c + jr, 32*m + b] = one DVE 32x32 block transpose.
  x is pre-transposed to stationary layout on host and DMAed in chunks.
"""

import os
import sys

import numpy as np

for _p in ("/opt/trn_rl_repo", "/root/.axon_site/_ro/trn_rl_repo"):
    if os.path.isdir(_p) and _p not in sys.path:
        sys.path.append(_p)

import ml_dtypes

BF16NP = ml_dtypes.bfloat16

B, T, H = 256, 200, 512
N_CORES = 8
BL = B // N_CORES  # 32
CHUNK = 8          # timesteps per x DMA chunk
NCHUNK = T // CHUNK

_CACHE = {}


# ---------------------------------------------------------------------------
# Host-side weight preparation (pure numpy, exact rearrangements)
# ---------------------------------------------------------------------------

def _arrange_w(W, xside):
    """[3H, H] (out, in) -> [128, 4, 3H] K-tile-arranged weight blocks.

    Partition p = 32*c_in + jr holds input dim h_in = 128*c_in + 32*m + jr for
    K-tile m. Free f = c_out*384 + slot*128 + j. PSUM layout: bank A is
    [gin | r | z] (slots 0,1,2 for the x side = gates n,r,z), bank B is [hn].
    The h side keeps slot order (r,z,n): its rz window is cols [128:384) of
    each 384 block, its n window is cols [256:384) -> bank B.
    """
    A = W.T.reshape(4, 4, 32, 3 * H)                # [c_in, m, jr, out]
    A = A.transpose(1, 0, 2, 3).reshape(4, 128, 3 * H)
    A = A.reshape(4, 128, 3, 4, 128)                # [m, p, gate(r,z,n), c, j]
    order = (2, 0, 1) if xside else (0, 1, 2)
    A = A[:, :, order, :, :]
    A = A.transpose(0, 1, 3, 2, 4).reshape(4, 128, 3 * H)
    A = A.transpose(1, 0, 2)                        # [p, m, out]
    return np.ascontiguousarray(A)


def _bias_A(bih, bhh):
    """[3H],[3H] -> [128, 384] f32 broadcast tile for bank A = [gin | r | z].

    gin gets bih_n; r and z get bih+bhh (both sides' matmuls accumulate in A).
    """
    rz = (bih + bhh)
    r = rz[0:512].reshape(4, 128)
    z = rz[512:1024].reshape(4, 128)
    gin = bih[1024:1536].reshape(4, 128)
    v = np.concatenate([gin, r, z], axis=1)         # [4, 384] per c block
    return np.repeat(v, 32, axis=0)                 # [128, 384]


def _bias_B(bhh):
    """[3H] -> [128, 128] f32 broadcast tile for bank B (bhh_n per c block)."""
    return np.repeat(bhh[1024:1536].reshape(4, 128), 32, axis=0)


def _arrange_x(x):
    """[BL, nt, H] -> [nt, 128, 128] stationary-layout bf16 (p=32c+jr, f=32m+b)."""
    nt = x.shape[1]
    xt = x.reshape(BL, nt, 4, 4, 32).transpose(1, 2, 4, 3, 0).reshape(nt, 128, 128)
    return np.ascontiguousarray(xt.astype(BF16NP))


# ---------------------------------------------------------------------------
# Bass program
# ---------------------------------------------------------------------------

def _build_program(n_steps=T, repeat=1, xwrap=None):
    """repeat>1 wraps the scan in a hardware loop; xwrap=N makes step t read
    x[t % N] so long timing variants reuse the same x buffer. Both are timing
    tools only (numerics are only meaningful for the default arguments)."""
    import concourse.bacc as bacc
    import concourse.tile as tile
    from concourse import mybir
    import concourse.bass as bass_mod
    from contextlib import ExitStack, nullcontext

    F32 = mybir.dt.float32
    BF16 = mybir.dt.bfloat16
    Sigmoid = mybir.ActivationFunctionType.Sigmoid
    Tanh = mybir.ActivationFunctionType.Tanh
    MULT = mybir.AluOpType.mult
    ADD = mybir.AluOpType.add

    xsteps = xwrap if xwrap is not None else n_steps
    nchunk = (xsteps + CHUNK - 1) // CHUNK
    nc = bacc.Bacc("TRN2", target_bir_lowering=False, debug=False)

    xt_dram = nc.declare_dram_parameter("xt", [xsteps, 128, 128], BF16, isOutput=False)
    w_dram = {
        name: nc.declare_dram_parameter(name, [128, 4, 3 * H], BF16, isOutput=False)
        for name in ("wgi", "wgh", "wai", "wah")
    }
    bA_dram = {
        "biasA_g": nc.declare_dram_parameter("biasA_g", [128, 384], F32, isOutput=False),
        "biasA_a": nc.declare_dram_parameter("biasA_a", [128, 384], F32, isOutput=False),
    }
    # hn-bias broadcast tiles for the B banks (DVE-prewritten, off-path)
    bB_dram = {
        "biasB_g": nc.declare_dram_parameter("biasB_g", [128, 128], F32, isOutput=False),
        "biasB_a": nc.declare_dram_parameter("biasB_a", [128, 128], F32, isOutput=False),
    }
    out = nc.declare_dram_parameter("out", [BL, H], F32, isOutput=True)

    with tile.TileContext(nc) as tc, ExitStack() as ctx:
        wpool = ctx.enter_context(tc.tile_pool(name="weights", bufs=1))
        st_pool = ctx.enter_context(tc.tile_pool(name="states", bufs=4))
        tmp_pool = ctx.enter_context(tc.tile_pool(name="tmps", bufs=3))
        ps_giA = ctx.enter_context(tc.tile_pool(name="psgiA", bufs=2, space="PSUM"))
        ps_giB = ctx.enter_context(tc.tile_pool(name="psgiB", bufs=2, space="PSUM"))
        ps_aiA = ctx.enter_context(tc.tile_pool(name="psaiA", bufs=2, space="PSUM"))
        ps_aiB = ctx.enter_context(tc.tile_pool(name="psaiB", bufs=2, space="PSUM"))

        # --- constants: weights + biases + x chunks ---
        wsb, bsb, xt_sb = {}, {}, []

        def dma_w(name, eng=None):
            t = wpool.tile([128, 4 * 3 * H], BF16, tag=name, name=name)
            (eng or nc.sync).dma_start(
                out=t, in_=w_dram[name][:].rearrange("p m f -> p (m f)"))
            wsb[name] = t

        def dma_bA(name, eng=None):
            drm = bA_dram[name]
            t = wpool.tile([128, 384], F32, tag=name, name=name)
            (eng or nc.sync).dma_start(out=t, in_=drm[:])
            bsb[name] = t

        def dma_x(ch):
            t0 = ch * CHUNK
            t1 = min(t0 + CHUNK, xsteps)
            t = wpool.tile([128, (t1 - t0) * 128], BF16, tag=f"xt{ch}", name=f"xt{ch}")
            src = bass_mod.AP(
                tensor=xt_dram[:].tensor,
                offset=t0 * 128 * 128,
                ap=[[128, 128], [128 * 128, t1 - t0], [1, 128]],
            )
            nc.sync.dma_start(out=t, in_=src)
            xt_sb.append(t)

        bsb_B = {}
        for cell, drm in (("g", bB_dram["biasB_g"]), ("a", bB_dram["biasB_a"])):
            t = wpool.tile([128, 128], F32, tag=f"biasB_{cell}", name=f"biasB_{cell}")
            nc.scalar.dma_start(out=t, in_=drm[:])
            bsb_B[cell] = t

        # split input DMAs across the two HWDGE queues (SP + ACT) so the
        # prologue weight transfers overlap ~2x
        dma_bA("biasA_g", nc.scalar)
        dma_w("wgi")
        dma_w("wgh", nc.scalar)
        dma_x(0)
        dma_w("wai", nc.scalar)
        dma_w("wah")
        dma_bA("biasA_a", nc.scalar)
        for ch in range(1, nchunk):
            dma_x(ch)

        def x_lhsT(t_, k):
            ch, off = divmod(t_ % xsteps, CHUNK)
            return xt_sb[ch][:, off * 128 + 32 * k: off * 128 + 32 * k + 32]

        def mm_x(psumA, lhsT_fn, w):
            """x-side group: accumulates A[0:384] = (gin|r|z) onto the
            engine-prewritten bias; all start=False."""
            for k in range(4):
                lhsT = lhsT_fn(k)
                for c in range(4):
                    base = k * 1536 + 384 * c
                    nc.tensor.matmul(
                        out=psumA[32 * c:32 * c + 32, 0:384],
                        lhsT=lhsT,
                        rhs=w[:, base:base + 384],
                        start=False, stop=False,
                        skip_group_check=True, tile_position=(0, 32 * c),
                    )

        def mm_h_rz(psumA, statT, w):
            """Hidden-side rz accumulate onto bank A [128:384]."""
            for k in range(4):
                lhsT = statT[:, 32 * k:32 * k + 32]
                for c in range(4):
                    base = k * 1536 + 384 * c
                    nc.tensor.matmul(
                        out=psumA[32 * c:32 * c + 32, 128:384],
                        lhsT=lhsT,
                        rhs=w[:, base:base + 256],
                        start=False, stop=(k == 3),
                        skip_group_check=True, tile_position=(0, 32 * c),
                    )

        def mm_h_hn(psumB, statT, w):
            """Hidden-side hn accumulate onto bank B [0:128] (over bias)."""
            for k in range(4):
                lhsT = statT[:, 32 * k:32 * k + 32]
                for c in range(4):
                    base = k * 1536 + 384 * c
                    nc.tensor.matmul(
                        out=psumB[32 * c:32 * c + 32, 0:128],
                        lhsT=lhsT,
                        rhs=w[:, base + 256:base + 384],
                        start=False, stop=(k == 3),
                        skip_group_check=True, tile_position=(0, 32 * c),
                    )

        def mm_hn_ain(psumB, psumA_ai, statT, w_h, w_ai):
            """Fused per-k sweep over the same stationary: ghid hn -> bank B,
            then ain -> the a-cell's A bank. One pass over statT instead of
            two, keeping the hn k3 completion (v_g's gate) unchanged."""
            for k in range(4):
                lhsT = statT[:, 32 * k:32 * k + 32]
                for c in range(4):
                    base = k * 1536 + 384 * c
                    nc.tensor.matmul(
                        out=psumB[32 * c:32 * c + 32, 0:128],
                        lhsT=lhsT,
                        rhs=w_h[:, base + 256:base + 384],
                        start=False, stop=(k == 3),
                        skip_group_check=True, tile_position=(0, 32 * c),
                    )
                for c in range(4):
                    base = k * 1536 + 384 * c
                    nc.tensor.matmul(
                        out=psumA_ai[32 * c:32 * c + 32, 0:384],
                        lhsT=lhsT,
                        rhs=w_ai[:, base:base + 384],
                        start=False, stop=False,
                        skip_group_check=True, tile_position=(0, 32 * c),
                    )

        def mm_h_fused(psumA, psumB, statT, w):
            """Per-k fused hidden sweep (rz -> A, hn -> B); one pass over
            statT. Used for the a cell, whose chain has slack for the later
            rz k3 completion."""
            for k in range(4):
                lhsT = statT[:, 32 * k:32 * k + 32]
                for c in range(4):
                    base = k * 1536 + 384 * c
                    nc.tensor.matmul(
                        out=psumA[32 * c:32 * c + 32, 128:384],
                        lhsT=lhsT,
                        rhs=w[:, base:base + 256],
                        start=False, stop=(k == 3),
                        skip_group_check=True, tile_position=(0, 32 * c),
                    )
                for c in range(4):
                    base = k * 1536 + 384 * c
                    nc.tensor.matmul(
                        out=psumB[32 * c:32 * c + 32, 0:128],
                        lhsT=lhsT,
                        rhs=w[:, base + 256:base + 384],
                        start=False, stop=(k == 3),
                        skip_group_check=True, tile_position=(0, 32 * c),
                    )

        # --- element-wise helpers ---
        def prewrite_A(pool, cell, tag):
            """New A bank pre-filled with [gin|r|z] biases (ACT, off-path)."""
            p = pool.tile([128, 512], F32, tag=tag, name=tag)
            nc.scalar.copy(p[:, 0:384], bsb["biasA_" + cell])
            return p

        def prewrite_B(pool, cell, tag):
            """New B bank pre-filled with the bhh_n bias (off-path).

            The g-cell's copy runs on ACT (which has slack) to relieve DVE,
            the busiest engine; the a-cell's stays on DVE."""
            p = pool.tile([128, 512], F32, tag=tag, name=tag)
            if cell == "g":
                nc.scalar.copy(p[:, 0:128], bsb_B[cell])
            else:
                nc.vector.tensor_copy(p[:, 0:128], bsb_B[cell])
            return p

        def sig_rz(pA, tag):
            rz = tmp_pool.tile([128, 256], F32, tag=tag + "rz", name="rz")
            nc.scalar.activation(rz, pA[:, 128:384], Sigmoid)
            return rz

        def v_of(rz, pB, tag):
            v = tmp_pool.tile([128, 128], F32, tag=tag + "v", name="v")
            nc.vector.tensor_mul(v, rz[:, 0:128], pB[:, 0:128])
            return v

        def t3_of(v, pA, tag):
            t3 = tmp_pool.tile([128, 128], F32, tag=tag + "t3", name="t3")
            nc.vector.tensor_add(t3, v, pA[:, 0:128])
            return t3

        def tanh_of(t3, tag):
            n = tmp_pool.tile([128, 128], BF16, tag=tag + "n", name="n")
            nc.scalar.activation(n, t3, Tanh)
            return n

        def omz_of(rz, tag):
            omz = tmp_pool.tile([128, 128], BF16, tag=tag + "omz", name="omz")
            nc.gpsimd.tensor_scalar(omz, rz[:, 128:256], -1.0, 1.0, MULT, ADD)
            return omz

        def zh_of(rz, row_prev, tag):
            zh = tmp_pool.tile([128, 128], BF16, tag=tag + "zh", name="zh")
            nc.gpsimd.tensor_mul(zh, rz[:, 128:256], row_prev)
            return zh

        def tail_of(omz, zh, n, tag, eng):
            row = st_pool.tile([128, 128], BF16, tag=tag + "row", name="row")
            if zh is None:
                eng.tensor_mul(row, omz, n)
            else:
                m1 = tmp_pool.tile([128, 128], BF16, tag=tag + "m1", name="m1")
                eng.tensor_mul(m1, omz, n)
                eng.tensor_add(row, m1, zh)
            return row

        def tr_of(row, tag):
            rowT = st_pool.tile([128, 128], BF16, tag=tag + "rowT", name="rowT")
            nc.vector.transpose(rowT, row)
            return rowT

        # --- pipeline state ---
        pgiA, pgiB = {}, {}
        paiA, paiB = {}, {}
        g_row, gT = {}, {}
        a_row, aT = {}, {}
        a_state = {}   # i -> (rz_a, v_a, t3_a, n_a, omz_a, zh_a) transient

        # ---- prologue ----
        pgiA[0] = prewrite_A(ps_giA, "g", "giA")
        mm_x(pgiA[0], lambda k: x_lhsT(0, k), wsb["wgi"])
        pgiB[0] = prewrite_B(ps_giB, "g", "giB")
        if n_steps > 1:
            pgiA[1] = prewrite_A(ps_giA, "g", "giA")
            mm_x(pgiA[1], lambda k: x_lhsT(1, k), wsb["wgi"])
            pgiB[1] = prewrite_B(ps_giB, "g", "giB")

        # g(0) chain (no hidden side, no zh)
        rz0 = sig_rz(pgiA[0], "g")
        v0 = v_of(rz0, pgiB[0], "g")
        t30 = t3_of(v0, pgiA[0], "g")
        n0 = tanh_of(t30, "g")
        omz0 = omz_of(rz0, "g")
        g_row[0] = tail_of(omz0, None, n0, "g", nc.vector)
        gT[0] = tr_of(g_row[0], "g")
        if n_steps > 2:
            pgiA[2] = prewrite_A(ps_giA, "g", "giA")
        paiA[0] = prewrite_A(ps_aiA, "a", "aiA")
        paiB[0] = prewrite_B(ps_aiB, "a", "aiB")

        # ---- steady-state scan ----
        # Per-engine stream order per macro-step i (producing g(i+1), a(i-1)
        # chain tails, and the a(i) matmuls):
        #   PE  : ghid_rz(i+1) | biasB_g+ghid_hn(i+1) | ain(i) | xproj(i+2)
        #         | biasB_a(i)+ahid(i)
        #   ACT : sig_a(i-1) | sig_g(i+1) | tanh_a(i-1) | tanh_g(i+1)
        #         | prewrite_aiA(i+1) | prewrite_giA(i+3)
        #   DVE : v_a,t3_a(i-1) | v_g,t3_g(i+1) | m1_a,row_a,trT_a(i-1)
        #         | m1_g,row_g,trT_g(i+1)
        #   POOL: omz_a,zh_a(i-1) | omz_g,zh_g(i+1)
        def emit_scan():
            for i in range(n_steps + 1):
                # 1. ACT: sig_a(i-1)
                if i >= 1:
                    rz_a = sig_rz(paiA[i - 1], "a")
                # 2. PE: ghid_rz(i+1) -> giA
                if i + 1 < n_steps:
                    mm_h_rz(pgiA[i + 1], gT[i], wsb["wgh"])
                # 3. DVE: v_a, t3_a (i-1)
                if i >= 1:
                    v_a = v_of(rz_a, paiB[i - 1], "a")
                    t3_a = t3_of(v_a, paiA[i - 1], "a")
                # 4. POOL: omz_a, zh_a (i-1)
                if i >= 1:
                    omz_a = omz_of(rz_a, "a")
                    zh_a = zh_of(rz_a, a_row[i - 2], "a") if i >= 2 else None
                # 5+7. PE: fused ghid_hn(i+1) + ain(i) sweep over gT(i)
                if i + 1 < n_steps:
                    mm_hn_ain(pgiB[i + 1], paiA[i], gT[i], wsb["wgh"],
                              wsb["wai"])
                elif i < n_steps:
                    mm_x(paiA[i], lambda k, i=i: gT[i][:, 32 * k:32 * k + 32],
                         wsb["wai"])
                # 6. ACT: sig_g(i+1)
                if i + 1 < n_steps:
                    rz_g = sig_rz(pgiA[i + 1], "g")
                # 8. DVE: v_g, t3_g (i+1)
                if i + 1 < n_steps:
                    v_g = v_of(rz_g, pgiB[i + 1], "g")
                    t3_g = t3_of(v_g, pgiA[i + 1], "g")
                # 9. ACT: tanh_a(i-1)
                if i >= 1:
                    n_a = tanh_of(t3_a, "a")
                # 10. POOL: omz_g, zh_g (i+1)
                if i + 1 < n_steps:
                    omz_g = omz_of(rz_g, "g")
                    zh_g = zh_of(rz_g, g_row[i], "g")
                # 11. PE: xproj(i+2) -> giA (prewritten at step i-1)
                if i + 2 < n_steps:
                    mm_x(pgiA[i + 2], lambda k, s=i + 2: x_lhsT(s, k), wsb["wgi"])
                # 12. DVE: m1_a, row_a, trT_a (i-1)
                if i >= 1:
                    a_row[i - 1] = tail_of(omz_a, zh_a, n_a, "a", nc.vector)
                    a_row.pop(i - 3, None)
                    if i < n_steps:
                        aT[i - 1] = tr_of(a_row[i - 1], "a")
                # 13. ACT: tanh_g(i+1)
                if i + 1 < n_steps:
                    n_g = tanh_of(t3_g, "g")
                # 14. DVE: m1_g, row_g, trT_g (i+1)
                if i + 1 < n_steps:
                    g_row[i + 1] = tail_of(omz_g, zh_g, n_g, "g", nc.vector)
                    gT[i + 1] = tr_of(g_row[i + 1], "g")
                    del pgiA[i + 1]
                    del pgiB[i + 1]
                    g_row.pop(i - 1, None)
                    gT.pop(i, None)
                # 15. PE: ahid(i), per-k fused rz+hn (aiB prewritten with bias)
                if 1 <= i < n_steps:
                    mm_h_fused(paiA[i], paiB[i], aT[i - 1], wsb["wah"])
                    aT.pop(i - 1, None)
                # 16. ACT: prewrite aiA(i+1)
                if i + 1 < n_steps:
                    paiA[i + 1] = prewrite_A(ps_aiA, "a", "aiA")
                # 17. ACT: prewrite giA(i+3) (for xproj(i+3) emitted next step)
                if i + 3 < n_steps:
                    pgiA[i + 3] = prewrite_A(ps_giA, "g", "giA")
                # 18. DVE: prewrite giB(i+2), aiB(i+1) (off-path, banks WAR-free)
                if i + 2 < n_steps:
                    pgiB[i + 2] = prewrite_B(ps_giB, "g", "giB")
                if i + 1 < n_steps:
                    paiB[i + 1] = prewrite_B(ps_aiB, "a", "aiB")
                # cleanup consumed psum refs for the a cell
                if i >= 1:
                    paiA.pop(i - 1, None)
                    paiB.pop(i - 1, None)

        loop_cm = tc.For_i(0, repeat, 1) if repeat > 1 else nullcontext()
        with loop_cm:
            emit_scan()

        # epilogue: final AUGRU state -> fp32 -> DRAM
        final = a_row[n_steps - 1]
        out_row = tmp_pool.tile([128, 128], F32, tag="outrow")
        nc.scalar.copy(out_row, final)
        out_ap = bass_mod.AP(
            tensor=out[:].tensor,
            offset=0,
            ap=[[128, 4], [H, BL], [1, 128]],
        )
        nc.sync.dma_start(out=out_ap, in_=out_row)

    nc.compile()
    return nc


def _get_program(n_steps=T):
    key = ("prog", n_steps)
    if key not in _CACHE:
        _CACHE[key] = _build_program(n_steps)
    return _CACHE[key]


# ---------------------------------------------------------------------------
# Entry point
# ---------------------------------------------------------------------------

def _make_consts(inputs):
    augru_Wih = np.asarray(inputs["augru_Wih"], np.float64)
    A1 = augru_Wih[:, :H]
    A2 = augru_Wih[:, H:]
    w_fused = A1 + A2 @ np.asarray(inputs["v_W"], np.float64)
    b_ai = np.asarray(inputs["augru_bih"], np.float64) + A2 @ np.asarray(
        inputs["v_b"], np.float64
    )
    b_ah = np.asarray(inputs["augru_bhh"], np.float64)
    gru_bih = np.asarray(inputs["gru_bih"], np.float64)
    gru_bhh = np.asarray(inputs["gru_bhh"], np.float64)

    def f32(x):
        return np.ascontiguousarray(x, dtype=np.float32)

    def bf(x):
        return np.ascontiguousarray(x.astype(np.float32)).astype(BF16NP)

    return {
        "wgi": bf(_arrange_w(np.asarray(inputs["gru_Wih"], np.float64), True)),
        "wgh": bf(_arrange_w(np.asarray(inputs["gru_Whh"], np.float64), False)),
        "wai": bf(_arrange_w(w_fused, True)),
        "wah": bf(_arrange_w(np.asarray(inputs["augru_Whh"], np.float64), False)),
        "biasA_g": f32(_bias_A(gru_bih, gru_bhh)),
        "biasA_a": f32(_bias_A(b_ai, b_ah)),
        "biasB_g": f32(_bias_B(gru_bhh)),
        "biasB_a": f32(_bias_B(b_ah)),
    }


def _make_in_maps(inputs):
    seq_emb = np.asarray(inputs["seq_emb"], np.float32)
    consts = _make_consts(inputs)
    return [
        {"xt": _arrange_x(seq_emb[c * BL:(c + 1) * BL]), **consts}
        for c in range(N_CORES)
    ]


def _prep_and_run(trace=False, **inputs):
    from concourse.bass_utils import run_bass_kernel_spmd

    in_maps = _make_in_maps(inputs)
    nc = _get_program()
    res = run_bass_kernel_spmd(nc, in_maps, list(range(N_CORES)), trace=trace)
    out = np.concatenate([res.results[c]["out"] for c in range(N_CORES)], axis=0)
    return out.astype(np.float32), res


def kernel(**inputs):
    out, _ = _prep_and_run(**inputs)
    return out


def kernel_traced(**inputs):
    """Like kernel() but profiles the run; returns (output, BassKernelResults)."""
    return _prep_and_run(**inputs, trace=True)
